# revision 4
# baseline (speedup 1.0000x reference)
"""Trainium2 Bass kernel for nn_DKL_45810121179236 (retrieval_knn).

Reference computation:
    C = cos_sim_matrix(ex, ey)            # [8192, 8192], D=256
    out1 = -sum(exp(c1)*c1), c1 = logN(1 - rowmax(C))
    out2 = -sum(exp(c2)*c2), c2 = logN(1 - colmax(C))

Sharding: ex rows split across 8 NeuronCores (1024 rows each); each core
computes its [1024, 8192] tile of C against the full ey, producing exact
local row-maxes and partial col-maxes. Host combines: concat row-maxes,
8-way elementwise max of col-max partials, then the two O(N) entropy sums.

v2 pipeline (vs baseline: fp8 DoubleRow matmul, cast-on-DMA loads,
norm-scale folded into PE transposes via diag matmul, 4x-mode DVE
reductions, ACT/DVE split PSUM egress):
  load:   gpsimd software-DGE DMA casts f32 HBM -> bf16 SBUF
  norm:   sumsq per 256-slice (DVE stt 4x, bf16), sqrt (ACT),
          reciprocal (DVE), diag(r) built in fp16 (DVE tensor_scalar)
  transp: regular matmul ybf_chunk^T @ diag(r) -> PSUM f32 (scaled
          transpose, 1 cyc/row); ACT copies PSUM -> fp8e4 DoubleRow tiles
  mm:     [128, 2, 128] x [128, 2, 512] fp8 DoubleRow matmuls, K=256 in
          one instruction at 0.5 cyc/row
  reduce: ACT/DVE split egress PSUM f32 -> bf16; col-max and per-x-tile
          row-max accumulated with scalar_tensor_tensor (4x mode)
  fold:   per-batch PE transpose of colacc + DVE reduce_max
"""

import sys

sys.path.insert(0, "/opt/trn_rl_repo")

import copy
from contextlib import ExitStack

import numpy as np

import concourse.bass as bass
import concourse.tile as tile
from concourse import mybir
from concourse import bass_utils
from concourse.masks import make_identity

N_CORES = 8
N = 8192  # ey rows (and total ex rows)
D = 256  # embedding dim
XR = N // N_CORES  # ex rows per core (1024)
NT_X = XR // 128  # 8 x-tiles per core
NB_Y = 8  # y load batches (1024 rows / 1024 j-columns each)
JB = 1024  # j-columns per batch

F32 = mybir.dt.float32
BF16 = mybir.dt.bfloat16
F16 = mybir.dt.float16
FP8 = mybir.dt.float8e4
AF = mybir.ActivationFunctionType
ALU = mybir.AluOpType
AX = mybir.AxisListType
DR = mybir.MatmulPerfMode.DoubleRow

SIGMA = 0.3

# x-tiles whose egress copy runs on DVE instead of ACT (load balance)
DVE_EGRESS_XT = (3, 7)


def _split_multi_waits(nc, max_waits=1):
    """The walrus build in this container rejects instructions carrying more
    than one sync wait. Move excess waits onto preceding same-engine NOPs
    (waits on one engine are sequential, so semantics are unchanged)."""
    n_split = 0
    for function in nc.m.functions:
        new_blocks = []
        for block in function.blocks:
            new_insts = []
            for inst in block.instructions:
                si = inst.sync_info
                if si is not None and si.on_wait and len(si.on_wait) > max_waits:
                    waits = list(si.on_wait)
                    n_split += 1
                    head, rest = waits[:-max_waits], waits[-max_waits:]
                    for ci in range(0, len(head), max_waits):
                        new_insts.append(
                            mybir.InstNoOp(
                                name=f"{inst.name}-ws{ci}",
                                engine=inst.engine,
                                sync_info=mybir.SyncInfo(
                                    on_wait=head[ci : ci + max_waits], on_update=[]
                                ),
                            )
                        )
                    inst = copy.replace(
                        inst,
                        sync_info=mybir.SyncInfo(
                            on_wait=rest, on_update=list(si.on_update)
                        ),
                    )
                new_insts.append(inst)
            new_blocks.append(copy.replace(block, instructions=new_insts))
        function.blocks.clear()
        for b in new_blocks:
            function.blocks.append(b)
    return n_split


def _emit_prep_batch(nc, pools, src_pqd, nq, ident, tT, col0):
    """Load nq*128 rows (cast to bf16 on the DMA), compute 1/||row||,
    and write normalized fp8 transposes into tT[:, half, col0 + ...]
    (DoubleRow K-subtile layout)."""
    raw = pools["raw"].tile([128, nq, D], BF16, tag="raw")
    nc.gpsimd.dma_start(raw[:], src_pqd)

    nsq = pools["sc"].tile([128, nq], F32, tag="sc")
    sq = pools["sq"].tile([128, D], BF16, tag="sq")
    for q in range(nq):
        nc.vector.scalar_tensor_tensor(
            sq[:],
            raw[:, q, :],
            1.0,
            raw[:, q, :],
            ALU.mult,
            ALU.mult,
            accum_out=nsq[:, q : q + 1],
        )
    nrm = pools["sc"].tile([128, nq], F32, tag="sc")
    nc.scalar.activation(nrm[:], nsq[:], AF.Sqrt)
    rns = pools["sc"].tile([128, nq], F32, tag="sc")
    nc.vector.reciprocal(rns[:], nrm[:])

    # scaled transposes: 2 q's (2 halves each) per [128, 512] PSUM tile
    for qq in range(0, nq, 2):
        dg0 = pools["dg"].tile([128, 128], F16, tag="dg")
        nc.vector.tensor_scalar(dg0[:], ident[:], rns[:, qq : qq + 1], None, ALU.mult)
        dg1 = pools["dg"].tile([128, 128], F16, tag="dg")
        nc.vector.tensor_scalar(
            dg1[:], ident[:], rns[:, qq + 1 : qq + 2], None, ALU.mult
        )
        ps = pools["pps"].tile([128, 4, 128], F32, tag="pps")
        for k, (q, dg) in enumerate(((qq, dg0), (qq + 1, dg1))):
            for half in range(2):
                nc.tensor.matmul(
                    ps[:, 2 * k + half, :],
                    raw[:, q, half * 128 : (half + 1) * 128],
                    dg[:],
                    start=True,
                    stop=True,
                )
        # psum layout [q, half, 128] -> dest [half, q, 128]
        nc.scalar.activation(
            tT[:, :, col0 + qq * 128 : col0 + (qq + 2) * 128].rearrange(
                "p h (q x) -> p h q x", q=2
            ),
            ps[:].rearrange("p (q h) x -> p h q x", q=2),
            AF.Copy,
        )


def _build():
    nc = bass.Bass("TRN2", target_bir_lowering=False, debug=False, num_devices=1)
    ex = nc.dram_tensor("ex_sh", [XR, D], F32, kind="ExternalInput").ap()
    ey = nc.dram_tensor("ey", [N, D], F32, kind="ExternalInput").ap()
    rowmax_o = nc.dram_tensor("rowmax", [XR], F32, kind="ExternalOutput").ap()
    colmax_o = nc.dram_tensor("colmax", [N], F32, kind="ExternalOutput").ap()

    with tile.TileContext(nc) as tc:
        with ExitStack() as ctx:
            ep = ctx.enter_context

            persist = ep(tc.tile_pool(name="persist", bufs=1))
            yT = persist.tile([128, 2, N], FP8, tag="yT")
            xT = persist.tile([128, 2, XR], FP8, tag="xT")
            colacc = persist.tile([128, N], BF16, tag="colacc")
            rowacc = persist.tile([128, NT_X, JB], BF16, tag="rowacc")
            rowmax_sb = persist.tile([128, NT_X], F32, tag="rowmax_sb")
            colmax_sb = persist.tile([128, N // 128], F32, tag="colmax_sb")
            ident_f16 = persist.tile([128, 128], F16, tag="ident_f16")
            ident_bf = persist.tile([128, 128], BF16, tag="ident_bf")
            ident_f32 = persist.tile([128, 128], F32, tag="ident_f32")
            make_identity(nc, ident_f16[:])
            make_identity(nc, ident_bf[:])
            make_identity(nc, ident_f32[:])

            pools = {
                "raw": ep(tc.tile_pool(name="raw", bufs=3)),
                "sq": ep(tc.tile_pool(name="sq", bufs=2)),
                "sc": ep(tc.tile_pool(name="sc", bufs=6)),
                "dg": ep(tc.tile_pool(name="dg", bufs=4)),
                "pps": ep(tc.tile_pool(name="pps", bufs=2, space="PSUM")),
                "fold": ep(tc.tile_pool(name="fold", bufs=1, space="PSUM")),
                "outp": ep(tc.tile_pool(name="outp", bufs=1, space="PSUM")),
            }
            mm_pool = ep(tc.tile_pool(name="mm", bufs=2, space="PSUM"))
            csb_pool = ep(tc.tile_pool(name="csb", bufs=4))
            out_pool = ep(tc.tile_pool(name="out", bufs=2))

            # ---- x prep (one batch of 1024 rows) ----
            xv = ex.rearrange("(q p) d -> p q d", p=128)
            _emit_prep_batch(nc, pools, xv, NT_X, ident_f16, xT, 0)

            yv = ey.rearrange("(b q p) d -> b p q d", p=128, q=8)
            for b in range(NB_Y):
                # ---- y batch prep ----
                _emit_prep_batch(nc, pools, yv[b], 8, ident_f16, yT, b * JB)

                # ---- matmuls + reductions for this j-batch ----
                j0 = b * JB
                cslice = colacc[:, j0 : j0 + JB]
                for xt in range(NT_X):
                    mm = mm_pool.tile([128, JB], F32, tag="mm")
                    for c in range(JB // 512):
                        nc.tensor.matmul(
                            mm[:, c * 512 : (c + 1) * 512],
                            xT[:, :, xt * 128 : (xt + 1) * 128],
                            yT[:, :, j0 + c * 512 : j0 + (c + 1) * 512],
                            start=True,
                            stop=True,
                            perf_mode=DR,
                        )
                    if xt == 0:
                        # egress doubles as colacc init; rowacc reads it
                        nc.scalar.activation(cslice, mm[:], AF.Copy)
                        src = cslice
                    else:
                        cbf = csb_pool.tile([128, JB], BF16, tag="csb")
                        if xt in DVE_EGRESS_XT:
                            nc.vector.tensor_copy(cbf[:], mm[:])
                        else:
                            nc.scalar.activation(cbf[:], mm[:], AF.Copy)
                        nc.vector.scalar_tensor_tensor(
                            cslice, cbf[:], 0.0, cslice, ALU.bypass, ALU.max
                        )
                        src = cbf[:]
                    ra = rowacc[:, xt, :]
                    if b == 0:
                        nc.vector.scalar_tensor_tensor(
                            ra, src, 0.0, src, ALU.bypass, ALU.max
                        )
                    else:
                        nc.vector.scalar_tensor_tensor(
                            ra, src, 0.0, ra, ALU.bypass, ALU.max
                        )

                # ---- col-max partition fold for this batch ----
                ps = pools["fold"].tile([128, 8, 128], BF16, tag="ppsb")
                for k in range(8):
                    nc.tensor.transpose(
                        ps[:, k, :],
                        colacc[:, j0 + k * 128 : j0 + (k + 1) * 128],
                        ident_bf[:],
                    )
                nc.vector.reduce_max(
                    colmax_sb[:, b * 8 : (b + 1) * 8], ps[:], axis=AX.X
                )

            # ---- row-max tail: fold rowacc per x-tile ----
            for xt in range(NT_X):
                half = out_pool.tile([128, 512], BF16, tag="rh")
                nc.vector.scalar_tensor_tensor(
                    half[:],
                    rowacc[:, xt, 0:512],
                    0.0,
                    rowacc[:, xt, 512:1024],
                    ALU.bypass,
                    ALU.max,
                )
                nc.vector.reduce_max(rowmax_sb[:, xt : xt + 1], half[:], axis=AX.X)

            # ---- outputs: transpose on PE so DMA writes are contiguous ----
            pso = pools["outp"].tile([128, 128], F32, tag="pps_o")
            nc.tensor.transpose(pso[0:NT_X, 0:128], rowmax_sb[:], ident_f32[:])
            rout = out_pool.tile([128, 128], F32, tag="out")
            nc.vector.tensor_copy(rout[0:NT_X, 0:128], pso[0:NT_X, 0:128])
            nc.sync.dma_start(rowmax_o.rearrange("(t p) -> t p", p=128), rout[0:NT_X, :])
            pso2 = pools["outp"].tile([128, 128], F32, tag="pps_o")
            nc.tensor.transpose(pso2[0:64, 0:128], colmax_sb[:], ident_f32[:])
            cout = out_pool.tile([128, 128], F32, tag="out")
            nc.vector.tensor_copy(cout[0:64, 0:128], pso2[0:64, 0:128])
            nc.sync.dma_start(colmax_o.rearrange("(c p) -> c p", p=128), cout[0:64, :])

    _split_multi_waits(nc)
    return nc


_NC_CACHE = []


def _get_nc():
    if not _NC_CACHE:
        _NC_CACHE.append(_build())
    return _NC_CACHE[0]


def run_device(ex, ey, trace=False):
    """Run the SPMD kernel; returns (rowmax [N], colmax [N], results obj)."""
    nc = _get_nc()
    in_maps = [
        {"ex_sh": np.ascontiguousarray(ex[k * XR : (k + 1) * XR]), "ey": ey}
        for k in range(N_CORES)
    ]
    res = bass_utils.run_bass_kernel_spmd(
        nc, in_maps, core_ids=list(range(N_CORES)), trace=trace
    )
    rowmax = np.concatenate([res.results[k]["rowmax"] for k in range(N_CORES)])
    colmax = np.max(
        np.stack([res.results[k]["colmax"] for k in range(N_CORES)]), axis=0
    )
    return rowmax, colmax, res


def _entropy(m):
    # -sum(exp(c)*c), c = logprob_Normal(1,SIGMA)(1 - m); accumulate in f64
    z = -m.astype(np.float64) / SIGMA
    c = -0.5 * z * z - np.log(SIGMA) - 0.5 * np.log(2.0 * np.pi)
    return -np.sum(np.exp(c) * c)


def kernel(ex, ey):
    ex = np.ascontiguousarray(np.asarray(ex), dtype=np.float32)
    ey = np.ascontiguousarray(np.asarray(ey), dtype=np.float32)
    rowmax, colmax, _ = run_device(ex, ey)
    out1 = np.float32(_entropy(rowmax))
    out2 = np.float32(_entropy(colmax))
    return (np.asarray(out1, dtype=np.float32), np.asarray(out2, dtype=np.float32))


# revision 7
# speedup vs baseline: 1.3157x; 1.3157x over previous
"""Trainium2 Bass kernel for nn_DKL_45810121179236 (retrieval_knn).

Reference computation:
    C = cos_sim_matrix(ex, ey)            # [8192, 8192], D=256
    out1 = -sum(exp(c1)*c1), c1 = logN(1 - rowmax(C))
    out2 = -sum(exp(c2)*c2), c2 = logN(1 - colmax(C))

Sharding: ex rows split across 8 NeuronCores (1024 rows each); each core
computes its [1024, 8192] tile of C against the full ey, producing exact
local row-maxes and partial col-maxes. Host combines: concat row-maxes,
8-way elementwise max of col-max partials, then the two O(N) entropy sums.

v3 (vs baseline): y batches stream through the matmul sweep (prep of
batch b+1 overlaps matmuls/reductions of batch b) instead of a serial
prep-then-sweep structure; loads cast f32->bf16 on the software-DGE DMA
so sumsq/scale read half the bytes; engine assignment measured from
hardware traces (TENSOR_TENSOR gets the 2x bf16 mode, scalar_tensor_
tensor does not; fp8 DoubleRow matmul is slower than bf16 pairs):
  ACT: PSUM egress f32->bf16 (with +half-ulp bias: the HW cast
       truncates, which biased both outputs ~1.2e-2 low), per-row
       Square+accum sumsq, sqrt, normalize-scale+cast
  DVE: col-max and per-x-tile row-max TT-max chains (2x mode),
       transpose copy-outs (2x PSUM-bf16 reads), reciprocal, folds
  PE:  bf16 K=128-pair matmuls, transposes
  Pool: software-DGE cast DMAs
"""

import sys

sys.path.insert(0, "/opt/trn_rl_repo")

import copy
from contextlib import ExitStack

import numpy as np

import concourse.bass as bass
import concourse.tile as tile
from concourse import mybir
from concourse import bass_utils
from concourse.masks import make_identity

N_CORES = 8
N = 8192  # ey rows (and total ex rows)
D = 256  # embedding dim
XR = N // N_CORES  # ex rows per core (1024)
NT_X = XR // 128  # 8 x-tiles per core
NB_Y = 8  # y load batches (1024 rows / 1024 j-columns each)
JB = 1024  # j-columns per batch

F32 = mybir.dt.float32
BF16 = mybir.dt.bfloat16
AF = mybir.ActivationFunctionType
ALU = mybir.AluOpType
AX = mybir.AxisListType

SIGMA = 0.3
# ACT Copy PSUM->bf16 truncates; +half ulp of bf16 near C~0.25 recenters
EGRESS_BIAS = 1.15e-4


def _split_multi_waits(nc, max_waits=1):
    """The walrus build in this container rejects instructions carrying more
    than one sync wait. Move excess waits onto preceding same-engine NOPs
    (waits on one engine are sequential, so semantics are unchanged)."""
    n_split = 0
    for function in nc.m.functions:
        new_blocks = []
        for block in function.blocks:
            new_insts = []
            for inst in block.instructions:
                si = inst.sync_info
                if si is not None and si.on_wait and len(si.on_wait) > max_waits:
                    waits = list(si.on_wait)
                    n_split += 1
                    head, rest = waits[:-max_waits], waits[-max_waits:]
                    for ci in range(0, len(head), max_waits):
                        new_insts.append(
                            mybir.InstNoOp(
                                name=f"{inst.name}-ws{ci}",
                                engine=inst.engine,
                                sync_info=mybir.SyncInfo(
                                    on_wait=head[ci : ci + max_waits], on_update=[]
                                ),
                            )
                        )
                    inst = copy.replace(
                        inst,
                        sync_info=mybir.SyncInfo(
                            on_wait=rest, on_update=list(si.on_update)
                        ),
                    )
                new_insts.append(inst)
            new_blocks.append(copy.replace(block, instructions=new_insts))
        function.blocks.clear()
        for b in new_blocks:
            function.blocks.append(b)
    return n_split


def _emit_prep_batch(nc, pools, src_pqd, nq, ident_bf, tT_hi, tT_lo, col0):
    """Load nq*128 rows (cast to bf16 on the DMA), normalize, and write
    bf16 transposes into tT_hi/lo[:, col0:col0+nq*128]."""
    raw = pools["raw"].tile([128, nq, D], BF16, tag="raw")
    nc.gpsimd.dma_start(raw[:], src_pqd)

    nsq = pools["sc"].tile([128, nq], F32, tag="sc")
    sq = pools["sq"].tile([128, D], BF16, tag="sq")
    for q in range(nq):
        nc.vector.scalar_tensor_tensor(
            sq[:],
            raw[:, q, :],
            1.0,
            raw[:, q, :],
            ALU.mult,
            ALU.mult,
            accum_out=nsq[:, q : q + 1],
        )
    nrm = pools["sc"].tile([128, nq], F32, tag="sc")
    nc.scalar.activation(nrm[:], nsq[:], AF.Sqrt)
    rns = pools["sc"].tile([128, nq], F32, tag="sc")
    nc.vector.reciprocal(rns[:], nrm[:])

    ysc = pools["ysc"].tile([128, nq, D], BF16, tag="ysc")
    for q in range(nq):
        nc.scalar.activation(
            ysc[:, q, :], raw[:, q, :], AF.Copy, scale=rns[:, q : q + 1]
        )

    # transposes: 4 [128,128] per [128, 4, 128] bf16 PSUM tile, DVE 2x copyout
    for half, tT in ((0, tT_hi), (1, tT_lo)):
        for qq in range(0, nq, 4):
            ps = pools["pps"].tile([128, 4, 128], BF16, tag="pps")
            for k in range(4):
                q = qq + k
                nc.tensor.transpose(
                    ps[:, k, :],
                    ysc[:, q, half * 128 : (half + 1) * 128],
                    ident_bf[:],
                )
            w = 4 * 128
            nc.vector.tensor_copy(
                tT[:, col0 + qq * 128 : col0 + qq * 128 + w],
                ps[:].rearrange("p k x -> p (k x)"),
            )


def _build():
    nc = bass.Bass("TRN2", target_bir_lowering=False, debug=False, num_devices=1)
    ex = nc.dram_tensor("ex_sh", [XR, D], F32, kind="ExternalInput").ap()
    ey = nc.dram_tensor("ey", [N, D], F32, kind="ExternalInput").ap()
    rowmax_o = nc.dram_tensor("rowmax", [XR], F32, kind="ExternalOutput").ap()
    colmax_o = nc.dram_tensor("colmax", [N], F32, kind="ExternalOutput").ap()

    with tile.TileContext(nc) as tc:
        with ExitStack() as ctx:
            ep = ctx.enter_context

            persist = ep(tc.tile_pool(name="persist", bufs=1))
            yT_hi = persist.tile([128, N], BF16, tag="yT_hi")
            yT_lo = persist.tile([128, N], BF16, tag="yT_lo")
            xT_hi = persist.tile([128, XR], BF16, tag="xT_hi")
            xT_lo = persist.tile([128, XR], BF16, tag="xT_lo")
            colacc = persist.tile([128, N], BF16, tag="colacc")
            rowacc = persist.tile([128, NT_X, JB], BF16, tag="rowacc")
            rowmax_sb = persist.tile([128, NT_X], F32, tag="rowmax_sb")
            colmax_sb = persist.tile([128, N // 128], F32, tag="colmax_sb")
            ident_bf = persist.tile([128, 128], BF16, tag="ident_bf")
            ident_f32 = persist.tile([128, 128], F32, tag="ident_f32")
            make_identity(nc, ident_bf[:])
            make_identity(nc, ident_f32[:])

            pools = {
                "raw": ep(tc.tile_pool(name="raw", bufs=3)),
                "ysc": ep(tc.tile_pool(name="ysc", bufs=2)),
                "sq": ep(tc.tile_pool(name="sq", bufs=2)),
                "sc": ep(tc.tile_pool(name="sc", bufs=6)),
                "pps": ep(tc.tile_pool(name="pps", bufs=1, space="PSUM")),
                "fold": ep(tc.tile_pool(name="fold", bufs=1, space="PSUM")),
            }
            mm_pool = ep(tc.tile_pool(name="mm", bufs=3, space="PSUM"))
            csb_pool = ep(tc.tile_pool(name="csb", bufs=4))
            out_pool = ep(tc.tile_pool(name="out", bufs=2))

            # ---- x prep (one batch of 1024 rows) ----
            xv = ex.rearrange("(q p) d -> p q d", p=128)
            _emit_prep_batch(nc, pools, xv, NT_X, ident_bf, xT_hi, xT_lo, 0)

            yv = ey.rearrange("(b q p) d -> b p q d", p=128, q=8)
            for b in range(NB_Y):
                # ---- y batch prep ----
                _emit_prep_batch(nc, pools, yv[b], 8, ident_bf, yT_hi, yT_lo, b * JB)

                # ---- matmuls + reductions for this j-batch ----
                j0 = b * JB
                cslice = colacc[:, j0 : j0 + JB]
                for xt in range(NT_X):
                    xs = slice(xt * 128, (xt + 1) * 128)
                    mm = mm_pool.tile([128, JB], F32, tag="mm")
                    for c in range(JB // 512):
                        pslice = mm[:, c * 512 : (c + 1) * 512]
                        js = slice(j0 + c * 512, j0 + (c + 1) * 512)
                        nc.tensor.matmul(
                            pslice, xT_hi[:, xs], yT_hi[:, js],
                            start=True, stop=False,
                        )
                        nc.tensor.matmul(
                            pslice, xT_lo[:, xs], yT_lo[:, js],
                            start=False, stop=True,
                        )
                    if xt == 0:
                        # egress doubles as colacc init; rowacc reads it
                        nc.scalar.activation(
                            cslice, mm[:], AF.Copy, bias=EGRESS_BIAS
                        )
                        src = cslice
                    else:
                        cbf = csb_pool.tile([128, JB], BF16, tag="csb")
                        if xt in (3, 7):
                            nc.vector.tensor_scalar(
                                cbf[:], mm[:], EGRESS_BIAS, None, ALU.add
                            )
                        else:
                            nc.scalar.activation(
                                cbf[:], mm[:], AF.Copy, bias=EGRESS_BIAS
                            )
                        nc.vector.tensor_max(cslice, cslice, cbf[:])
                        src = cbf[:]
                    ra = rowacc[:, xt, :]
                    if b == 0:
                        nc.vector.tensor_copy(ra, src)
                    else:
                        nc.vector.tensor_max(ra, ra, src)

                # ---- col-max partition fold for this batch ----
                ps = pools["fold"].tile([128, 8, 128], BF16, tag="ppsb")
                for k in range(8):
                    nc.tensor.transpose(
                        ps[:, k, :],
                        colacc[:, j0 + k * 128 : j0 + (k + 1) * 128],
                        ident_bf[:],
                    )
                nc.vector.reduce_max(
                    colmax_sb[:, b * 8 : (b + 1) * 8], ps[:], axis=AX.X
                )

            # ---- row-max tail: fold rowacc per x-tile ----
            for xt in range(NT_X):
                nc.vector.reduce_max(
                    rowmax_sb[:, xt : xt + 1], rowacc[:, xt, :], axis=AX.X
                )

            # ---- outputs: transpose on PE so DMA writes are contiguous ----
            pso = pools["fold"].tile([128, 128], F32, tag="ppsb")
            nc.tensor.transpose(pso[0:NT_X, 0:128], rowmax_sb[:], ident_f32[:])
            rout = out_pool.tile([128, 128], F32, tag="out")
            nc.vector.tensor_copy(rout[0:NT_X, 0:128], pso[0:NT_X, 0:128])
            nc.sync.dma_start(rowmax_o.rearrange("(t p) -> t p", p=128), rout[0:NT_X, :])
            pso2 = pools["fold"].tile([128, 128], F32, tag="ppsb")
            nc.tensor.transpose(pso2[0:64, 0:128], colmax_sb[:], ident_f32[:])
            cout = out_pool.tile([128, 128], F32, tag="out")
            nc.vector.tensor_copy(cout[0:64, 0:128], pso2[0:64, 0:128])
            nc.sync.dma_start(colmax_o.rearrange("(c p) -> c p", p=128), cout[0:64, :])

    _split_multi_waits(nc)
    return nc


_NC_CACHE = []


def _get_nc():
    if not _NC_CACHE:
        _NC_CACHE.append(_build())
    return _NC_CACHE[0]


def run_device(ex, ey, trace=False):
    """Run the SPMD kernel; returns (rowmax [N], colmax [N], results obj)."""
    nc = _get_nc()
    in_maps = [
        {"ex_sh": np.ascontiguousarray(ex[k * XR : (k + 1) * XR]), "ey": ey}
        for k in range(N_CORES)
    ]
    res = bass_utils.run_bass_kernel_spmd(
        nc, in_maps, core_ids=list(range(N_CORES)), trace=trace
    )
    rowmax = np.concatenate([res.results[k]["rowmax"] for k in range(N_CORES)])
    colmax = np.max(
        np.stack([res.results[k]["colmax"] for k in range(N_CORES)]), axis=0
    )
    return rowmax, colmax, res


def _entropy(m):
    # -sum(exp(c)*c), c = logprob_Normal(1,SIGMA)(1 - m); accumulate in f64
    z = -m.astype(np.float64) / SIGMA
    c = -0.5 * z * z - np.log(SIGMA) - 0.5 * np.log(2.0 * np.pi)
    return -np.sum(np.exp(c) * c)


def kernel(ex, ey):
    ex = np.ascontiguousarray(np.asarray(ex), dtype=np.float32)
    ey = np.ascontiguousarray(np.asarray(ey), dtype=np.float32)
    rowmax, colmax, _ = run_device(ex, ey)
    out1 = np.float32(_entropy(rowmax))
    out2 = np.float32(_entropy(colmax))
    return (np.asarray(out1, dtype=np.float32), np.asarray(out2, dtype=np.float32))


# revision 8
# speedup vs baseline: 1.4306x; 1.0873x over previous
"""Trainium2 Bass kernel for nn_DKL_45810121179236 (retrieval_knn).

Reference computation:
    C = cos_sim_matrix(ex, ey)            # [8192, 8192], D=256
    out1 = -sum(exp(c1)*c1), c1 = logN(1 - rowmax(C))
    out2 = -sum(exp(c2)*c2), c2 = logN(1 - colmax(C))

Sharding: ex rows split across 8 NeuronCores (1024 rows each); each core
computes its [1024, 8192] tile of C against the full ey, producing exact
local row-maxes and partial col-maxes. Host combines: concat row-maxes,
8-way elementwise max of col-max partials, then the two O(N) entropy sums.

Per-core pipeline (engines in parens):
  load:   1MB batches, 1024 rows as [128, (8, 256)] (sync HWDGE)
  norm:   per 256-slice sum-of-squares (ACT Square+accum), sqrt (ACT),
          reciprocal (DVE), scale+cast bf16 (ACT Copy w/ scale AP)
  transp: PE transpose 128x128 into PSUM, 4 per bank, copy out (DVE)
  mm:     [128, 1024] PSUM groups; 2x(N=512) x 2 K-chunk bf16 matmuls (PE)
  reduce: PSUM->SBUF bf16 copy (ACT); col-max acc TT-max (DVE, 2x mode);
          row-max chain TT-max (DVE)
  fold:   PE-transpose col-max acc, 3-D reduce_max (DVE)
"""

import sys

sys.path.insert(0, "/opt/trn_rl_repo")

import copy
from contextlib import ExitStack

import numpy as np

import concourse.bass as bass
import concourse.tile as tile
from concourse import mybir
from concourse import bass_utils
from concourse.masks import make_identity

N_CORES = 8
N = 8192  # ey rows (and total ex rows)
D = 256  # embedding dim
XR = N // N_CORES  # ex rows per core (1024)
NT_X = XR // 128  # 8 x-tiles per core
NT_Y = N // 128  # 64 y-tiles
NB_Y = 8  # y load batches (1024 rows each)
JG = 1024  # j-group width = 2 PSUM banks
NG = N // JG  # 8 j-groups per x-tile

F32 = mybir.dt.float32
BF16 = mybir.dt.bfloat16
AF = mybir.ActivationFunctionType
ALU = mybir.AluOpType
AX = mybir.AxisListType

SIGMA = 0.3


def _split_multi_waits(nc, max_waits=1):
    """The walrus build in this container rejects instructions carrying more
    than one sync wait. Move excess waits onto preceding same-engine NOPs
    (waits on one engine are sequential, so semantics are unchanged)."""
    n_split = 0
    for function in nc.m.functions:
        new_blocks = []
        for block in function.blocks:
            new_insts = []
            for inst in block.instructions:
                si = inst.sync_info
                if si is not None and si.on_wait and len(si.on_wait) > max_waits:
                    waits = list(si.on_wait)
                    n_split += 1
                    head, rest = waits[:-max_waits], waits[-max_waits:]
                    for ci in range(0, len(head), max_waits):
                        new_insts.append(
                            mybir.InstNoOp(
                                name=f"{inst.name}-ws{ci}",
                                engine=inst.engine,
                                sync_info=mybir.SyncInfo(
                                    on_wait=head[ci : ci + max_waits], on_update=[]
                                ),
                            )
                        )
                    inst = copy.replace(
                        inst,
                        sync_info=mybir.SyncInfo(
                            on_wait=rest, on_update=list(si.on_update)
                        ),
                    )
                new_insts.append(inst)
            new_blocks.append(copy.replace(block, instructions=new_insts))
        function.blocks.clear()
        for b in new_blocks:
            function.blocks.append(b)
    return n_split


def _emit_prep_batch(nc, pools, src_pqd, b, nq, tT_hi, tT_lo, ident, rns_out=None):
    """Load nq*128 rows, normalize, cast bf16, PE-transpose into tT_hi/lo
    columns [b*128*nq, ...). src_pqd is a [p, q, d] DRAM view of this batch.
    With rns_out, the scale step is skipped (folded downstream) and the
    reciprocal norms are stored there instead."""
    raw = pools["raw"].tile([128, nq * D], F32, tag="raw")
    nc.sync.dma_start(raw[:].rearrange("p (q d) -> p q d", q=nq), src_pqd)

    nsq = pools["sc"].tile([128, nq], F32, tag="sc")
    sq = pools["sq"].tile([128, D], F32, tag="sq")
    for q in range(nq):
        nc.vector.scalar_tensor_tensor(
            sq[:],
            raw[:, q * D : (q + 1) * D],
            1.0,
            raw[:, q * D : (q + 1) * D],
            ALU.mult,
            ALU.mult,
            accum_out=nsq[:, q : q + 1],
        )
    nrm = pools["sc"].tile([128, nq], F32, tag="sc")
    nc.scalar.activation(nrm[:], nsq[:], AF.Sqrt)
    rns = rns_out if rns_out is not None else pools["sc"].tile(
        [128, nq], F32, tag="sc"
    )
    nc.vector.reciprocal(rns[:], nrm[:])

    ybf = pools["bf"].tile([128, nq * D], BF16, tag="bf")
    for q in range(nq):
        if rns_out is None:
            # scale+cast on ACT: out = Copy(in * rns[q])
            nc.scalar.activation(
                ybf[:, q * D : (q + 1) * D],
                raw[:, q * D : (q + 1) * D],
                AF.Copy,
                scale=rns[:, q : q + 1],
            )
        else:
            # x side: plain cast; 1/||x_i|| folds into the PSUM->SBUF copies
            nc.scalar.activation(
                ybf[:, q * D : (q + 1) * D], raw[:, q * D : (q + 1) * D], AF.Copy
            )

    # PE transposes: 4 per [128, 512] psum tile, then one copy per tile
    for half, tT in ((0, tT_hi), (1, tT_lo)):
        for qq in range(0, nq, 4):
            ps = pools["pps"].tile([128, 512], BF16, tag="pps")
            for q in range(qq, min(qq + 4, nq)):
                nc.tensor.transpose(
                    ps[:, (q - qq) * 128 : (q - qq + 1) * 128],
                    ybf[:, q * D + half * 128 : q * D + half * 128 + 128],
                    ident[:],
                )
            w = (min(qq + 4, nq) - qq) * 128
            col0 = b * nq * 128 + qq * 128
            nc.vector.tensor_copy(tT[:, col0 : col0 + w], ps[:, 0:w])


def _build():
    nc = bass.Bass("TRN2", target_bir_lowering=False, debug=False, num_devices=1)
    ex = nc.dram_tensor("ex_sh", [XR, D], F32, kind="ExternalInput").ap()
    ey = nc.dram_tensor("ey", [N, D], F32, kind="ExternalInput").ap()
    rowmax_o = nc.dram_tensor("rowmax", [XR], F32, kind="ExternalOutput").ap()
    colmax_o = nc.dram_tensor("colmax", [N], F32, kind="ExternalOutput").ap()

    with tile.TileContext(nc) as tc:
        with ExitStack() as ctx:
            ep = ctx.enter_context

            persist = ep(tc.tile_pool(name="persist", bufs=1))
            yT_hi = persist.tile([128, N], BF16, tag="yT_hi")
            yT_lo = persist.tile([128, N], BF16, tag="yT_lo")
            xT_hi = persist.tile([128, XR], BF16, tag="xT_hi")
            xT_lo = persist.tile([128, XR], BF16, tag="xT_lo")
            colacc = persist.tile([128, N], BF16, tag="colacc")
            rowmax_sb = persist.tile([128, NT_X], F32, tag="rowmax_sb")
            colmax_sb = persist.tile([128, NT_Y], F32, tag="colmax_sb")
            rx_sb = persist.tile([128, NT_X], F32, tag="rx_sb")
            ident_bf = persist.tile([128, 128], BF16, tag="ident_bf")
            ident_f32 = persist.tile([128, 128], F32, tag="ident_f32")
            make_identity(nc, ident_bf[:])
            make_identity(nc, ident_f32[:])

            pools = {
                "raw": ep(tc.tile_pool(name="raw", bufs=3)),
                "sq": ep(tc.tile_pool(name="sq", bufs=2)),
                "sc": ep(tc.tile_pool(name="sc", bufs=9)),
                "bf": ep(tc.tile_pool(name="bf", bufs=3)),
                "pps": ep(tc.tile_pool(name="pps", bufs=2, space="PSUM")),
            }
            mm_pool = ep(tc.tile_pool(name="mm", bufs=3, space="PSUM"))
            csb_pool = ep(tc.tile_pool(name="csb", bufs=4))
            row_pool = ep(tc.tile_pool(name="row", bufs=2))
            out_pool = ep(tc.tile_pool(name="out", bufs=2))

            # ---- prep: x (one batch), then y (8 batches) ----
            xv = ex.rearrange("(q p) d -> p q d", p=128)
            _emit_prep_batch(nc, pools, xv, 0, NT_X, xT_hi, xT_lo, ident_bf)
            yv = ey.rearrange("(b q p) d -> b p q d", p=128, q=8)
            for b in range(NB_Y):
                _emit_prep_batch(nc, pools, yv[b], b, 8, yT_hi, yT_lo, ident_bf)

            # ---- matmul sweep + reductions ----
            for xt in range(NT_X):
                rowacc = row_pool.tile([128, JG], BF16, tag="row")
                xs = slice(xt * 128, (xt + 1) * 128)
                for g in range(NG):
                    ps = mm_pool.tile([128, JG], F32, tag="mm")
                    for c in range(JG // 512):
                        j0 = g * JG + c * 512
                        pslice = ps[:, c * 512 : (c + 1) * 512]
                        nc.tensor.matmul(
                            pslice,
                            xT_hi[:, xs],
                            yT_hi[:, j0 : j0 + 512],
                            start=True,
                            stop=False,
                        )
                        nc.tensor.matmul(
                            pslice,
                            xT_lo[:, xs],
                            yT_lo[:, j0 : j0 + 512],
                            start=False,
                            stop=True,
                        )
                    c_sb = csb_pool.tile([128, JG], BF16, tag="csb")
                    nc.scalar.activation(c_sb[:], ps[:], AF.Copy)
                    # col-max accumulate across x-tiles
                    acc_slice = colacc[:, g * JG : (g + 1) * JG]
                    if xt == 0:
                        nc.vector.tensor_copy(acc_slice, c_sb[:])
                    else:
                        nc.vector.tensor_max(acc_slice, acc_slice, c_sb[:])
                    # row-max chain within this x-tile (group width)
                    if g == 0:
                        nc.vector.tensor_copy(rowacc[:], c_sb[:])
                    else:
                        nc.vector.tensor_max(rowacc[:], rowacc[:], c_sb[:])
                nc.vector.reduce_max(
                    rowmax_sb[:, xt : xt + 1], rowacc[:], axis=AX.X
                )

            # ---- col-max partition fold ----
            for fg in range(NT_Y // 4):
                ps = pools["pps"].tile([128, 512], BF16, tag="pps")
                for k in range(4):
                    cch = fg * 4 + k
                    nc.tensor.transpose(
                        ps[:, k * 128 : (k + 1) * 128],
                        colacc[:, cch * 128 : (cch + 1) * 128],
                        ident_bf[:],
                    )
                nc.vector.reduce_max(
                    colmax_sb[:, fg * 4 : (fg + 1) * 4],
                    ps[:].rearrange("p (k q) -> p k q", k=4),
                    axis=AX.X,
                )

            # ---- outputs: transpose on PE so DMA writes are contiguous ----
            pso = pools["pps"].tile([128, 128], F32, tag="pps")
            # rowmax [128, 8] -> [8, 128]
            nc.tensor.transpose(pso[0:8, 0:128], rowmax_sb[:], ident_f32[:])
            rout = out_pool.tile([128, 128], F32, tag="out")
            nc.vector.tensor_copy(rout[0:8, 0:128], pso[0:8, 0:128])
            nc.sync.dma_start(rowmax_o.rearrange("(t p) -> t p", p=128), rout[0:8, :])
            # colmax [128, 64] -> [64, 128]
            pso2 = pools["pps"].tile([128, 128], F32, tag="pps")
            nc.tensor.transpose(pso2[0:64, 0:128], colmax_sb[:], ident_f32[:])
            cout = out_pool.tile([128, 128], F32, tag="out")
            nc.vector.tensor_copy(cout[0:64, 0:128], pso2[0:64, 0:128])
            nc.sync.dma_start(colmax_o.rearrange("(c p) -> c p", p=128), cout[0:64, :])

    _split_multi_waits(nc)
    return nc


_NC_CACHE = []


def _get_nc():
    if not _NC_CACHE:
        _NC_CACHE.append(_build())
    return _NC_CACHE[0]


def run_device(ex, ey, trace=False):
    """Run the SPMD kernel; returns (rowmax [N], colmax [N], results obj)."""
    nc = _get_nc()
    in_maps = [
        {"ex_sh": np.ascontiguousarray(ex[k * XR : (k + 1) * XR]), "ey": ey}
        for k in range(N_CORES)
    ]
    res = bass_utils.run_bass_kernel_spmd(
        nc, in_maps, core_ids=list(range(N_CORES)), trace=trace
    )
    rowmax = np.concatenate([res.results[k]["rowmax"] for k in range(N_CORES)])
    colmax = np.max(
        np.stack([res.results[k]["colmax"] for k in range(N_CORES)]), axis=0
    )
    return rowmax, colmax, res


def _entropy(m):
    # -sum(exp(c)*c), c = logprob_Normal(1,SIGMA)(1 - m); accumulate in f64
    z = -m.astype(np.float64) / SIGMA
    c = -0.5 * z * z - np.log(SIGMA) - 0.5 * np.log(2.0 * np.pi)
    return -np.sum(np.exp(c) * c)


def kernel(ex, ey):
    ex = np.ascontiguousarray(np.asarray(ex), dtype=np.float32)
    ey = np.ascontiguousarray(np.asarray(ey), dtype=np.float32)
    rowmax, colmax, _ = run_device(ex, ey)
    out1 = np.float32(_entropy(rowmax))
    out2 = np.float32(_entropy(colmax))
    return (np.asarray(out1, dtype=np.float32), np.asarray(out2, dtype=np.float32))



# revision 9
# speedup vs baseline: 1.4519x; 1.0149x over previous
"""Trainium2 Bass kernel for nn_DKL_45810121179236 (retrieval_knn).

Reference computation:
    C = cos_sim_matrix(ex, ey)            # [8192, 8192], D=256
    out1 = -sum(exp(c1)*c1), c1 = logN(1 - rowmax(C))
    out2 = -sum(exp(c2)*c2), c2 = logN(1 - colmax(C))

Sharding: ex rows split across 8 NeuronCores (1024 rows each); each core
computes its [1024, 8192] tile of C against the full ey, producing exact
local row-maxes and partial col-maxes. Host combines: concat row-maxes,
8-way elementwise max of col-max partials, then the two O(N) entropy sums.

Per-core pipeline (engines in parens):
  load:   1MB batches, 1024 rows as [128, (8, 256)] (sync HWDGE)
  norm:   per 256-slice sum-of-squares (ACT Square+accum), sqrt (ACT),
          reciprocal (DVE), scale+cast bf16 (ACT Copy w/ scale AP)
  transp: PE transpose 128x128 into PSUM, 4 per bank, copy out (DVE)
  mm:     [128, 1024] PSUM groups; 2x(N=512) x 2 K-chunk bf16 matmuls (PE)
  reduce: PSUM->SBUF bf16 copy (ACT); col-max acc TT-max (DVE, 2x mode);
          row-max chain TT-max (DVE)
  fold:   PE-transpose col-max acc, 3-D reduce_max (DVE)
"""

import sys

sys.path.insert(0, "/opt/trn_rl_repo")

import copy
from contextlib import ExitStack

import numpy as np

import concourse.bass as bass
import concourse.tile as tile
from concourse import mybir
from concourse import bass_utils
from concourse.masks import make_identity

N_CORES = 8
N = 8192  # ey rows (and total ex rows)
D = 256  # embedding dim
XR = N // N_CORES  # ex rows per core (1024)
NT_X = XR // 128  # 8 x-tiles per core
NT_Y = N // 128  # 64 y-tiles
NB_Y = 8  # y load batches (1024 rows each)
JG = 1024  # j-group width = 2 PSUM banks
NG = N // JG  # 8 j-groups per x-tile

F32 = mybir.dt.float32
BF16 = mybir.dt.bfloat16
AF = mybir.ActivationFunctionType
ALU = mybir.AluOpType
AX = mybir.AxisListType

SIGMA = 0.3


def _split_multi_waits(nc, max_waits=1):
    """The walrus build in this container rejects instructions carrying more
    than one sync wait. Move excess waits onto preceding same-engine NOPs
    (waits on one engine are sequential, so semantics are unchanged)."""
    n_split = 0
    for function in nc.m.functions:
        new_blocks = []
        for block in function.blocks:
            new_insts = []
            for inst in block.instructions:
                si = inst.sync_info
                if si is not None and si.on_wait and len(si.on_wait) > max_waits:
                    waits = list(si.on_wait)
                    n_split += 1
                    head, rest = waits[:-max_waits], waits[-max_waits:]
                    for ci in range(0, len(head), max_waits):
                        new_insts.append(
                            mybir.InstNoOp(
                                name=f"{inst.name}-ws{ci}",
                                engine=inst.engine,
                                sync_info=mybir.SyncInfo(
                                    on_wait=head[ci : ci + max_waits], on_update=[]
                                ),
                            )
                        )
                    inst = copy.replace(
                        inst,
                        sync_info=mybir.SyncInfo(
                            on_wait=rest, on_update=list(si.on_update)
                        ),
                    )
                new_insts.append(inst)
            new_blocks.append(copy.replace(block, instructions=new_insts))
        function.blocks.clear()
        for b in new_blocks:
            function.blocks.append(b)
    return n_split


def _emit_prep_batch(nc, pools, src_pqd, b, nq, tT_hi, tT_lo, ident, rns_out=None):
    """Load nq*128 rows, normalize, cast bf16, PE-transpose into tT_hi/lo
    columns [b*128*nq, ...). src_pqd is a [p, q, d] DRAM view of this batch.
    With rns_out, the scale step is skipped (folded downstream) and the
    reciprocal norms are stored there instead."""
    raw = pools["raw"].tile([128, nq * D], F32, tag="raw")
    nc.sync.dma_start(raw[:].rearrange("p (q d) -> p q d", q=nq), src_pqd)

    nsq = pools["sc"].tile([128, nq], F32, tag="sc")
    sq = pools["sq"].tile([128, D], F32, tag="sq")
    for q in range(nq):
        nc.vector.scalar_tensor_tensor(
            sq[:],
            raw[:, q * D : (q + 1) * D],
            1.0,
            raw[:, q * D : (q + 1) * D],
            ALU.mult,
            ALU.mult,
            accum_out=nsq[:, q : q + 1],
        )
    nrm = pools["sc"].tile([128, nq], F32, tag="sc")
    nc.scalar.activation(nrm[:], nsq[:], AF.Sqrt)
    rns = rns_out if rns_out is not None else pools["sc"].tile(
        [128, nq], F32, tag="sc"
    )
    nc.vector.reciprocal(rns[:], nrm[:])

    ybf = pools["bf"].tile([128, nq * D], BF16, tag="bf")
    for q in range(nq):
        if rns_out is None:
            # scale+cast on ACT: out = Copy(in * rns[q])
            nc.scalar.activation(
                ybf[:, q * D : (q + 1) * D],
                raw[:, q * D : (q + 1) * D],
                AF.Copy,
                scale=rns[:, q : q + 1],
            )
        else:
            # x side: plain cast; 1/||x_i|| folds into the PSUM->SBUF copies
            nc.scalar.activation(
                ybf[:, q * D : (q + 1) * D], raw[:, q * D : (q + 1) * D], AF.Copy
            )

    # PE transposes: 4 per [128, 512] psum tile, then one copy per tile
    for half, tT in ((0, tT_hi), (1, tT_lo)):
        for qq in range(0, nq, 4):
            ps = pools["pps"].tile([128, 512], BF16, tag="pps")
            for q in range(qq, min(qq + 4, nq)):
                nc.tensor.transpose(
                    ps[:, (q - qq) * 128 : (q - qq + 1) * 128],
                    ybf[:, q * D + half * 128 : q * D + half * 128 + 128],
                    ident[:],
                )
            w = (min(qq + 4, nq) - qq) * 128
            col0 = b * nq * 128 + qq * 128
            nc.vector.tensor_copy(tT[:, col0 : col0 + w], ps[:, 0:w])


def _build():
    nc = bass.Bass("TRN2", target_bir_lowering=False, debug=False, num_devices=1)
    ex = nc.dram_tensor("ex_sh", [XR, D], F32, kind="ExternalInput").ap()
    ey = nc.dram_tensor("ey", [N, D], F32, kind="ExternalInput").ap()
    rowmax_o = nc.dram_tensor("rowmax", [XR], F32, kind="ExternalOutput").ap()
    colmax_o = nc.dram_tensor("colmax", [N], F32, kind="ExternalOutput").ap()

    with tile.TileContext(nc) as tc:
        with ExitStack() as ctx:
            ep = ctx.enter_context

            persist = ep(tc.tile_pool(name="persist", bufs=1))
            yT_hi = persist.tile([128, N], BF16, tag="yT_hi")
            yT_lo = persist.tile([128, N], BF16, tag="yT_lo")
            xT_hi = persist.tile([128, XR], BF16, tag="xT_hi")
            xT_lo = persist.tile([128, XR], BF16, tag="xT_lo")
            colacc = persist.tile([128, N], BF16, tag="colacc")
            rowmax_sb = persist.tile([128, NT_X], F32, tag="rowmax_sb")
            colmax_sb = persist.tile([128, NT_Y], F32, tag="colmax_sb")
            rx_sb = persist.tile([128, NT_X], F32, tag="rx_sb")
            ident_bf = persist.tile([128, 128], BF16, tag="ident_bf")
            ident_f32 = persist.tile([128, 128], F32, tag="ident_f32")
            make_identity(nc, ident_bf[:])
            make_identity(nc, ident_f32[:])

            pools = {
                "raw": ep(tc.tile_pool(name="raw", bufs=3)),
                "sq": ep(tc.tile_pool(name="sq", bufs=2)),
                "sc": ep(tc.tile_pool(name="sc", bufs=9)),
                "bf": ep(tc.tile_pool(name="bf", bufs=3)),
                "pps": ep(tc.tile_pool(name="pps", bufs=4, space="PSUM")),
            }
            csb_pool = ep(tc.tile_pool(name="csb", bufs=4))
            row_pool = ep(tc.tile_pool(name="row", bufs=2))
            out_pool = ep(tc.tile_pool(name="out", bufs=2))

            # ---- prep: x (one batch), then y (8 batches) ----
            xv = ex.rearrange("(q p) d -> p q d", p=128)
            _emit_prep_batch(nc, pools, xv, 0, NT_X, xT_hi, xT_lo, ident_bf)
            yv = ey.rearrange("(b q p) d -> b p q d", p=128, q=8)
            for b in range(NB_Y):
                _emit_prep_batch(nc, pools, yv[b], b, 8, yT_hi, yT_lo, ident_bf)

            # ---- matmul sweep + reductions ----
            for xt in range(NT_X):
                rowacc = row_pool.tile([128, JG], BF16, tag="row")
                xs = slice(xt * 128, (xt + 1) * 128)
                for g in range(NG):
                    ps = pools["pps"].tile([128, JG], F32, tag="pps")
                    for c in range(JG // 512):
                        j0 = g * JG + c * 512
                        pslice = ps[:, c * 512 : (c + 1) * 512]
                        nc.tensor.matmul(
                            pslice,
                            xT_hi[:, xs],
                            yT_hi[:, j0 : j0 + 512],
                            start=True,
                            stop=False,
                        )
                        nc.tensor.matmul(
                            pslice,
                            xT_lo[:, xs],
                            yT_lo[:, j0 : j0 + 512],
                            start=False,
                            stop=True,
                        )
                    c_sb = csb_pool.tile([128, JG], BF16, tag="csb")
                    nc.scalar.activation(c_sb[:], ps[:], AF.Copy)
                    # col-max accumulate across x-tiles
                    acc_slice = colacc[:, g * JG : (g + 1) * JG]
                    if xt == 0:
                        nc.vector.tensor_copy(acc_slice, c_sb[:])
                    else:
                        nc.vector.tensor_max(acc_slice, acc_slice, c_sb[:])
                    # row-max chain within this x-tile (group width)
                    if g == 0:
                        nc.vector.tensor_copy(rowacc[:], c_sb[:])
                    else:
                        nc.vector.tensor_max(rowacc[:], rowacc[:], c_sb[:])
                nc.vector.reduce_max(
                    rowmax_sb[:, xt : xt + 1], rowacc[:], axis=AX.X
                )

            # ---- col-max partition fold ----
            for fg in range(NT_Y // 4):
                ps = pools["pps"].tile([128, 512], BF16, tag="pps")
                for k in range(4):
                    cch = fg * 4 + k
                    nc.tensor.transpose(
                        ps[:, k * 128 : (k + 1) * 128],
                        colacc[:, cch * 128 : (cch + 1) * 128],
                        ident_bf[:],
                    )
                nc.vector.reduce_max(
                    colmax_sb[:, fg * 4 : (fg + 1) * 4],
                    ps[:].rearrange("p (k q) -> p k q", k=4),
                    axis=AX.X,
                )

            # ---- outputs: transpose on PE so DMA writes are contiguous ----
            pso = pools["pps"].tile([128, 128], F32, tag="pps")
            # rowmax [128, 8] -> [8, 128]
            nc.tensor.transpose(pso[0:8, 0:128], rowmax_sb[:], ident_f32[:])
            rout = out_pool.tile([128, 128], F32, tag="out")
            nc.vector.tensor_copy(rout[0:8, 0:128], pso[0:8, 0:128])
            nc.sync.dma_start(rowmax_o.rearrange("(t p) -> t p", p=128), rout[0:8, :])
            # colmax [128, 64] -> [64, 128]
            pso2 = pools["pps"].tile([128, 128], F32, tag="pps")
            nc.tensor.transpose(pso2[0:64, 0:128], colmax_sb[:], ident_f32[:])
            cout = out_pool.tile([128, 128], F32, tag="out")
            nc.vector.tensor_copy(cout[0:64, 0:128], pso2[0:64, 0:128])
            nc.sync.dma_start(colmax_o.rearrange("(c p) -> c p", p=128), cout[0:64, :])

    _split_multi_waits(nc)
    return nc


_NC_CACHE = []


def _get_nc():
    if not _NC_CACHE:
        _NC_CACHE.append(_build())
    return _NC_CACHE[0]


def run_device(ex, ey, trace=False):
    """Run the SPMD kernel; returns (rowmax [N], colmax [N], results obj)."""
    nc = _get_nc()
    in_maps = [
        {"ex_sh": np.ascontiguousarray(ex[k * XR : (k + 1) * XR]), "ey": ey}
        for k in range(N_CORES)
    ]
    res = bass_utils.run_bass_kernel_spmd(
        nc, in_maps, core_ids=list(range(N_CORES)), trace=trace
    )
    rowmax = np.concatenate([res.results[k]["rowmax"] for k in range(N_CORES)])
    colmax = np.max(
        np.stack([res.results[k]["colmax"] for k in range(N_CORES)]), axis=0
    )
    return rowmax, colmax, res


def _entropy(m):
    # -sum(exp(c)*c), c = logprob_Normal(1,SIGMA)(1 - m); accumulate in f64
    z = -m.astype(np.float64) / SIGMA
    c = -0.5 * z * z - np.log(SIGMA) - 0.5 * np.log(2.0 * np.pi)
    return -np.sum(np.exp(c) * c)


def kernel(ex, ey):
    ex = np.ascontiguousarray(np.asarray(ex), dtype=np.float32)
    ey = np.ascontiguousarray(np.asarray(ey), dtype=np.float32)
    rowmax, colmax, _ = run_device(ex, ey)
    out1 = np.float32(_entropy(rowmax))
    out2 = np.float32(_entropy(colmax))
    return (np.asarray(out1, dtype=np.float32), np.asarray(out2, dtype=np.float32))



# revision 10
# speedup vs baseline: 1.4872x; 1.0243x over previous
"""Trainium2 Bass kernel for nn_DKL_45810121179236 (retrieval_knn).

Reference computation:
    C = cos_sim_matrix(ex, ey)            # [8192, 8192], D=256
    out1 = -sum(exp(c1)*c1), c1 = logN(1 - rowmax(C))
    out2 = -sum(exp(c2)*c2), c2 = logN(1 - colmax(C))

Sharding: ex rows split across 8 NeuronCores (1024 rows each); each core
computes its [1024, 8192] tile of C against the full ey, producing exact
local row-maxes and partial col-maxes. Host combines: concat row-maxes,
8-way elementwise max of col-max partials, then the two O(N) entropy sums.

Per-core pipeline (engines in parens):
  load:   1MB batches, 1024 rows as [128, (8, 256)] (sync HWDGE)
  norm:   per 256-slice sum-of-squares (ACT Square+accum), sqrt (ACT),
          reciprocal (DVE), scale+cast bf16 (ACT Copy w/ scale AP)
  transp: PE transpose 128x128 into PSUM, 4 per bank, copy out (DVE)
  mm:     [128, 1024] PSUM groups; 2x(N=512) x 2 K-chunk bf16 matmuls (PE)
  reduce: PSUM->SBUF bf16 copy (ACT); col-max acc TT-max (DVE, 2x mode);
          row-max chain TT-max (DVE)
  fold:   PE-transpose col-max acc, 3-D reduce_max (DVE)
"""

import sys

sys.path.insert(0, "/opt/trn_rl_repo")

import copy
from contextlib import ExitStack

import numpy as np

import concourse.bass as bass
import concourse.tile as tile
from concourse import mybir
from concourse import bass_utils
from concourse.masks import make_identity

N_CORES = 8
N = 8192  # ey rows (and total ex rows)
D = 256  # embedding dim
XR = N // N_CORES  # ex rows per core (1024)
NT_X = XR // 128  # 8 x-tiles per core
NT_Y = N // 128  # 64 y-tiles
NB_Y = 8  # y load batches (1024 rows each)
JG = 1024  # j-group width = 2 PSUM banks
NG = N // JG  # 8 j-groups per x-tile

F32 = mybir.dt.float32
BF16 = mybir.dt.bfloat16
AF = mybir.ActivationFunctionType
ALU = mybir.AluOpType
AX = mybir.AxisListType

SIGMA = 0.3


def _split_multi_waits(nc, max_waits=1):
    """The walrus build in this container rejects instructions carrying more
    than one sync wait. Move excess waits onto preceding same-engine NOPs
    (waits on one engine are sequential, so semantics are unchanged)."""
    n_split = 0
    for function in nc.m.functions:
        new_blocks = []
        for block in function.blocks:
            new_insts = []
            for inst in block.instructions:
                si = inst.sync_info
                if si is not None and si.on_wait and len(si.on_wait) > max_waits:
                    waits = list(si.on_wait)
                    n_split += 1
                    head, rest = waits[:-max_waits], waits[-max_waits:]
                    for ci in range(0, len(head), max_waits):
                        new_insts.append(
                            mybir.InstNoOp(
                                name=f"{inst.name}-ws{ci}",
                                engine=inst.engine,
                                sync_info=mybir.SyncInfo(
                                    on_wait=head[ci : ci + max_waits], on_update=[]
                                ),
                            )
                        )
                    inst = copy.replace(
                        inst,
                        sync_info=mybir.SyncInfo(
                            on_wait=rest, on_update=list(si.on_update)
                        ),
                    )
                new_insts.append(inst)
            new_blocks.append(copy.replace(block, instructions=new_insts))
        function.blocks.clear()
        for b in new_blocks:
            function.blocks.append(b)
    return n_split


def _emit_prep_batch(nc, pools, src_pqd, b, nq, tT_hi, tT_lo, ident, rns_out=None):
    """Load nq*128 rows, normalize, cast bf16, PE-transpose into tT_hi/lo
    columns [b*128*nq, ...). src_pqd is a [p, q, d] DRAM view of this batch.
    With rns_out, the scale step is skipped (folded downstream) and the
    reciprocal norms are stored there instead."""
    raw = pools["raw"].tile([128, nq * D], F32, tag="raw")
    nc.sync.dma_start(raw[:].rearrange("p (q d) -> p q d", q=nq), src_pqd)

    nsq = pools["sc"].tile([128, nq], F32, tag="sc")
    sq = pools["sq"].tile([128, D], F32, tag="sq")
    for q in range(nq):
        nc.vector.scalar_tensor_tensor(
            sq[:],
            raw[:, q * D : (q + 1) * D],
            1.0,
            raw[:, q * D : (q + 1) * D],
            ALU.mult,
            ALU.mult,
            accum_out=nsq[:, q : q + 1],
        )
    nrm = pools["sc"].tile([128, nq], F32, tag="sc")
    nc.scalar.activation(nrm[:], nsq[:], AF.Sqrt)
    rns = rns_out if rns_out is not None else pools["sc"].tile(
        [128, nq], F32, tag="sc"
    )
    nc.vector.reciprocal(rns[:], nrm[:])

    ybf = pools["bf"].tile([128, nq * D], BF16, tag="bf")
    for q in range(nq):
        if rns_out is None:
            # scale+cast on ACT: out = Copy(in * rns[q])
            nc.scalar.activation(
                ybf[:, q * D : (q + 1) * D],
                raw[:, q * D : (q + 1) * D],
                AF.Copy,
                scale=rns[:, q : q + 1],
            )
        else:
            # x side: plain cast; 1/||x_i|| folds into the PSUM->SBUF copies
            nc.scalar.activation(
                ybf[:, q * D : (q + 1) * D], raw[:, q * D : (q + 1) * D], AF.Copy
            )

    # PE transposes: 4 per [128, 512] psum tile, then one copy per tile
    for half, tT in ((0, tT_hi), (1, tT_lo)):
        for qq in range(0, nq, 4):
            ps = pools["pps"].tile([128, 512], BF16, tag="pps")
            for q in range(qq, min(qq + 4, nq)):
                nc.tensor.transpose(
                    ps[:, (q - qq) * 128 : (q - qq + 1) * 128],
                    ybf[:, q * D + half * 128 : q * D + half * 128 + 128],
                    ident[:],
                )
            w = (min(qq + 4, nq) - qq) * 128
            col0 = b * nq * 128 + qq * 128
            if (qq // 4 + half) % 2 == 0:
                nc.scalar.activation(tT[:, col0 : col0 + w], ps[:, 0:w], AF.Copy)
            else:
                nc.vector.tensor_copy(tT[:, col0 : col0 + w], ps[:, 0:w])


def _build():
    nc = bass.Bass("TRN2", target_bir_lowering=False, debug=False, num_devices=1)
    ex = nc.dram_tensor("ex_sh", [XR, D], F32, kind="ExternalInput").ap()
    ey = nc.dram_tensor("ey", [N, D], F32, kind="ExternalInput").ap()
    rowmax_o = nc.dram_tensor("rowmax", [XR], F32, kind="ExternalOutput").ap()
    colmax_o = nc.dram_tensor("colmax", [N], F32, kind="ExternalOutput").ap()

    with tile.TileContext(nc) as tc:
        with ExitStack() as ctx:
            ep = ctx.enter_context

            persist = ep(tc.tile_pool(name="persist", bufs=1))
            yT_hi = persist.tile([128, N], BF16, tag="yT_hi")
            yT_lo = persist.tile([128, N], BF16, tag="yT_lo")
            xT_hi = persist.tile([128, XR], BF16, tag="xT_hi")
            xT_lo = persist.tile([128, XR], BF16, tag="xT_lo")
            colacc = persist.tile([128, N], BF16, tag="colacc")
            rowmax_sb = persist.tile([128, NT_X], F32, tag="rowmax_sb")
            colmax_sb = persist.tile([128, NT_Y], F32, tag="colmax_sb")
            rx_sb = persist.tile([128, NT_X], F32, tag="rx_sb")
            ident_bf = persist.tile([128, 128], BF16, tag="ident_bf")
            ident_f32 = persist.tile([128, 128], F32, tag="ident_f32")
            make_identity(nc, ident_bf[:])
            make_identity(nc, ident_f32[:])

            pools = {
                "raw": ep(tc.tile_pool(name="raw", bufs=3)),
                "sq": ep(tc.tile_pool(name="sq", bufs=2)),
                "sc": ep(tc.tile_pool(name="sc", bufs=9)),
                "bf": ep(tc.tile_pool(name="bf", bufs=3)),
                "pps": ep(tc.tile_pool(name="pps", bufs=4, space="PSUM")),
            }
            csb_pool = ep(tc.tile_pool(name="csb", bufs=4))
            row_pool = ep(tc.tile_pool(name="row", bufs=2))
            out_pool = ep(tc.tile_pool(name="out", bufs=2))

            # ---- prep: x (one batch), then y (8 batches) ----
            xv = ex.rearrange("(q p) d -> p q d", p=128)
            _emit_prep_batch(nc, pools, xv, 0, NT_X, xT_hi, xT_lo, ident_bf)
            yv = ey.rearrange("(b q p) d -> b p q d", p=128, q=8)
            for b in range(NB_Y):
                _emit_prep_batch(nc, pools, yv[b], b, 8, yT_hi, yT_lo, ident_bf)

            # ---- matmul sweep + reductions ----
            for xt in range(NT_X):
                rowacc = row_pool.tile([128, JG], BF16, tag="row")
                xs = slice(xt * 128, (xt + 1) * 128)
                for g in range(NG):
                    ps = pools["pps"].tile([128, JG], F32, tag="pps")
                    for c in range(JG // 512):
                        j0 = g * JG + c * 512
                        pslice = ps[:, c * 512 : (c + 1) * 512]
                        nc.tensor.matmul(
                            pslice,
                            xT_hi[:, xs],
                            yT_hi[:, j0 : j0 + 512],
                            start=True,
                            stop=False,
                        )
                        nc.tensor.matmul(
                            pslice,
                            xT_lo[:, xs],
                            yT_lo[:, j0 : j0 + 512],
                            start=False,
                            stop=True,
                        )
                    c_sb = csb_pool.tile([128, JG], BF16, tag="csb")
                    nc.scalar.activation(c_sb[:], ps[:], AF.Copy)
                    # col-max accumulate across x-tiles
                    acc_slice = colacc[:, g * JG : (g + 1) * JG]
                    if xt == 0:
                        nc.vector.tensor_copy(acc_slice, c_sb[:])
                    else:
                        nc.vector.tensor_max(acc_slice, acc_slice, c_sb[:])
                    # row-max chain within this x-tile (group width)
                    if g == 0:
                        nc.vector.tensor_copy(rowacc[:], c_sb[:])
                    else:
                        nc.vector.tensor_max(rowacc[:], rowacc[:], c_sb[:])
                nc.vector.reduce_max(
                    rowmax_sb[:, xt : xt + 1], rowacc[:], axis=AX.X
                )

            # ---- col-max partition fold ----
            for fg in range(NT_Y // 4):
                ps = pools["pps"].tile([128, 512], BF16, tag="pps")
                for k in range(4):
                    cch = fg * 4 + k
                    nc.tensor.transpose(
                        ps[:, k * 128 : (k + 1) * 128],
                        colacc[:, cch * 128 : (cch + 1) * 128],
                        ident_bf[:],
                    )
                nc.vector.reduce_max(
                    colmax_sb[:, fg * 4 : (fg + 1) * 4],
                    ps[:].rearrange("p (k q) -> p k q", k=4),
                    axis=AX.X,
                )

            # ---- outputs: transpose on PE so DMA writes are contiguous ----
            pso = pools["pps"].tile([128, 128], F32, tag="pps")
            # rowmax [128, 8] -> [8, 128]
            nc.tensor.transpose(pso[0:8, 0:128], rowmax_sb[:], ident_f32[:])
            rout = out_pool.tile([128, 128], F32, tag="out")
            nc.vector.tensor_copy(rout[0:8, 0:128], pso[0:8, 0:128])
            nc.sync.dma_start(rowmax_o.rearrange("(t p) -> t p", p=128), rout[0:8, :])
            # colmax [128, 64] -> [64, 128]
            pso2 = pools["pps"].tile([128, 128], F32, tag="pps")
            nc.tensor.transpose(pso2[0:64, 0:128], colmax_sb[:], ident_f32[:])
            cout = out_pool.tile([128, 128], F32, tag="out")
            nc.vector.tensor_copy(cout[0:64, 0:128], pso2[0:64, 0:128])
            nc.sync.dma_start(colmax_o.rearrange("(c p) -> c p", p=128), cout[0:64, :])

    _split_multi_waits(nc)
    return nc


_NC_CACHE = []


def _get_nc():
    if not _NC_CACHE:
        _NC_CACHE.append(_build())
    return _NC_CACHE[0]


def run_device(ex, ey, trace=False):
    """Run the SPMD kernel; returns (rowmax [N], colmax [N], results obj)."""
    nc = _get_nc()
    in_maps = [
        {"ex_sh": np.ascontiguousarray(ex[k * XR : (k + 1) * XR]), "ey": ey}
        for k in range(N_CORES)
    ]
    res = bass_utils.run_bass_kernel_spmd(
        nc, in_maps, core_ids=list(range(N_CORES)), trace=trace
    )
    rowmax = np.concatenate([res.results[k]["rowmax"] for k in range(N_CORES)])
    colmax = np.max(
        np.stack([res.results[k]["colmax"] for k in range(N_CORES)]), axis=0
    )
    return rowmax, colmax, res


def _entropy(m):
    # -sum(exp(c)*c), c = logprob_Normal(1,SIGMA)(1 - m); accumulate in f64
    z = -m.astype(np.float64) / SIGMA
    c = -0.5 * z * z - np.log(SIGMA) - 0.5 * np.log(2.0 * np.pi)
    return -np.sum(np.exp(c) * c)


def kernel(ex, ey):
    ex = np.ascontiguousarray(np.asarray(ex), dtype=np.float32)
    ey = np.ascontiguousarray(np.asarray(ey), dtype=np.float32)
    rowmax, colmax, _ = run_device(ex, ey)
    out1 = np.float32(_entropy(rowmax))
    out2 = np.float32(_entropy(colmax))
    return (np.asarray(out1, dtype=np.float32), np.asarray(out2, dtype=np.float32))

